# revision 25
# baseline (speedup 1.0000x reference)
"""Causal attention block kernel for TRN2, 8 NeuronCores.

Sharding: 8 cores = 4 batches x 2 head-groups (8 heads each).
Each core computes, for its (batch, head-group):
  qkv = x @ w_qkv + b_qkv ; causal softmax attention ; partial out-proj.
Host sums the two head-group partials per batch and adds b_out.

Schedule (v2): all weights arrive via large host-prearranged contiguous
DMAs.  Pair-0 Q/K projection is dc-pipelined against the streamed X^T
chunk loads across 8 PSUM banks, so the PE starts ~2us in.  V-projection
and the remaining Q/K pairs are deferred filler units threaded through
the attention chunk stream, which runs with a 2-chunk scores->PV skew so
exp (scalar engine) latency stays off the PE critical path.  opsum banks
drain per q-block as soon as their accumulation stops (kc == 4qb+3);
normalization (reciprocal + selector broadcast) chases the drains per
(pair, q-block), and the output projection chases head 7's drains so
only the last q-block's projection trails the attention stream.
"""

import numpy as np
from contextlib import ExitStack

import concourse.bacc as bacc
import concourse.bass as bass
import concourse.mybir as mybir
import concourse.tile as tile
from concourse import bass_utils

F32 = mybir.dt.float32
BF16 = mybir.dt.bfloat16
AF = mybir.ActivationFunctionType

B, S, D, H, DH = 4, 2048, 1024, 16, 64
HPC = 8            # heads per core
NP = 4             # head pairs per core
NS = S // 128      # 16 s-tiles / k-chunks
NQ = S // 512      # 4 q-blocks
NDC = D // 128     # 8 d-chunks


def _emit(ctx: ExitStack, tc: tile.TileContext, io):
    nc = tc.nc
    xt_d, wqk_d, bqk_d, wv_d, bv_d, wo_d, sel_d, tri_d, out_d = io

    const = ctx.enter_context(tc.tile_pool(name="const", bufs=1))

    # ---- weight/constant loads (all large contiguous DMAs, issue order =
    # priority: pair0 QK weights, X^T chunks, V weights, consts, rest) ----
    wqk = {}
    xt = [const.tile([128, S], BF16, tag=f"xt{dc}", name=f"xt{dc}") for dc in range(NDC)]
    wvt = const.tile([128, NDC * 512], BF16, tag="wv", name="wvt")
    # first pair-0 weight half + first xt half land first so the dc=0
    # matmuls can issue ~2.5us in; halves keep each transfer short.
    for t in range(2):
        mt = t * NP
        wqk[(0, t)] = const.tile([128, 1024], BF16, tag=f"wq{0}{t}", name=f"wqk0{t}")
        nc.sync.dma_start(wqk[(0, t)][:, 0:512], wqk_d[:, mt * 1024:mt * 1024 + 512])
    nc.sync.dma_start(xt[0][:, 0:1024], xt_d[:, 0:1024])
    nc.sync.dma_start(xt[0][:, 1024:S], xt_d[:, 1024:S])
    bqk = const.tile([128, 8], F32, tag="bqk", name="bqk")
    bv = const.tile([128, HPC * DH], F32, tag="bv", name="bv")
    tri = const.tile([128, 128], BF16, tag="tri", name="tri")
    sel = const.tile([128, NP * 128], BF16, tag="sel", name="sel")
    for dc in range(1, NDC):
        nc.sync.dma_start(xt[dc][:], xt_d[:, dc * S:(dc + 1) * S])
        if dc == 1:
            # dc4-7 pair-0 weight halves: needed only from the dc=4 matmuls
            for t in range(2):
                mt = t * NP
                nc.sync.dma_start(
                    wqk[(0, t)][:, 512:1024],
                    wqk_d[:, mt * 1024 + 512:(mt + 1) * 1024])
            nc.sync.dma_start(bqk[:], bqk_d[:])
        elif dc == 2:
            nc.sync.dma_start(bv[:], bv_d[:])
        elif dc == 3:
            nc.sync.dma_start(tri[:], tri_d[:])
    # V weights right after the x^T stream (V-proj consumes them after the
    # pair-0 dc pipeline finishes); halves so dc0-3 can start first
    nc.sync.dma_start(wvt[:, 0:2048], wv_d[:, 0:2048])
    nc.sync.dma_start(wvt[:, 2048:4096], wv_d[:, 2048:4096])
    nc.sync.dma_start(sel[:], sel_d[:])
    for j in range(1, NP):
        for t in range(2):
            mt = t * NP + j
            wqk[(j, t)] = const.tile([128, 1024], BF16, tag=f"wq{j}{t}", name=f"wqk{j}{t}")
            nc.sync.dma_start(wqk[(j, t)][:], wqk_d[:, mt * 1024:(mt + 1) * 1024])
    wot = const.tile([128, NP * 1024], BF16, tag="wo", name="wot")
    nc.sync.dma_start(wot[:], wo_d[:])

    # ---- persistent attention tensors ----
    qkt_pool = ctx.enter_context(tc.tile_pool(name="qkt", bufs=10))
    v3_pool = ctx.enter_context(tc.tile_pool(name="v3", bufs=1))
    oo_pool = ctx.enter_context(tc.tile_pool(name="oo", bufs=1))
    dn_pool = ctx.enter_context(tc.tile_pool(name="dn", bufs=1))
    e_pool = ctx.enter_context(tc.tile_pool(name="epool", bufs=5))
    drain_pool = ctx.enter_context(tc.tile_pool(name="drain", bufs=4))

    qt = [qkt_pool.tile([128, S], BF16, tag="qkt", name="qkt") for j in range(NP)]
    kt = [qkt_pool.tile([128, S], BF16, tag="qkt", name="qkt") for j in range(NP)]
    v3 = [v3_pool.tile([128, HPC * 65], BF16, tag=f"v3_{st}", name=f"v3_{st}")
          for st in range(NS)]
    oo = [oo_pool.tile([128, S], BF16, tag=f"oo{j}", name=f"oo{j}") for j in range(NP)]
    # head h's reciprocal denominator row lives at partition 32*(h//2) + h%2
    # (reciprocal happens on the drain staging tile; the DMA ships it here)
    recip = dn_pool.tile([128, S], BF16, tag="recip", name="recip")
    # selector matmul reads all 8 recip rows before late pairs are written;
    # 0*garbage must stay 0, so clear once (NaN guard).
    nc.vector.memset(recip[:], 1.0)
    # 1-element dummy activation: pulls the ACT table load to t~1us so the
    # first real ScalarE ops (pair-0 bias adds) aren't serialized behind it
    nc.scalar.activation(recip[0:1, 0:1], recip[0:1, 0:1], AF.Exp)

    # ---- phase 1: pair-0 Q/K projection, dc-pipelined over 8 psum banks ----
    with tc.tile_pool(name="psA", bufs=8, space="PSUM") as psA:
        ps0 = [psA.tile([128, 512], F32, tag="qk0", name="psqk0") for _ in range(8)]
        for dc in range(NDC):
            for nb in range(NQ):
                for ti in range(2):
                    nc.tensor.matmul(
                        ps0[ti * 4 + nb][:],
                        wqk[(0, ti)][:, dc * 128:(dc + 1) * 128],
                        xt[dc][:, nb * 512:(nb + 1) * 512],
                        start=(dc == 0), stop=(dc == NDC - 1),
                    )
        # bias adds split across DVE and ScalarE so neither serializes the
        # first score matmuls (which need the nb=0 windows of qt[0]/kt[0])
        for nb in range(NQ):
            for ti in range(2):
                mt = ti * NP
                dst = qt[0] if ti == 0 else kt[0]
                if nb % 2 == 0:
                    nc.vector.tensor_scalar_add(
                        dst[:, nb * 512:(nb + 1) * 512], ps0[ti * 4 + nb][:],
                        bqk[:, mt:mt + 1])
                else:
                    nc.scalar.activation(
                        dst[:, nb * 512:(nb + 1) * 512], ps0[ti * 4 + nb][:],
                        AF.Identity, bias=bqk[:, mt:mt + 1])

    # ---- phase 2: attention stream with filler units ----
    with (
        tc.tile_pool(name="pst", bufs=2, space="PSUM") as pst,
        tc.tile_pool(name="pso", bufs=4, space="PSUM") as pso,
    ):
        def v_unit(st):
            def emit():
                ps = pst.tile([128, 512], F32, tag="st", name="psv")
                for dc in range(NDC):
                    nc.tensor.matmul(
                        ps[:], xt[dc][:, st * 128:(st + 1) * 128],
                        wvt[:, dc * 512:(dc + 1) * 512],
                        start=(dc == 0), stop=(dc == NDC - 1),
                    )
                v_view = v3[st][:].rearrange("p (h e) -> p h e", h=HPC)[:, :, 0:DH]
                nc.vector.tensor_add(
                    v_view,
                    ps[:].rearrange("p (h e) -> p h e", h=HPC),
                    bv[:].rearrange("p (h e) -> p h e", h=HPC),
                )
                nc.vector.memset(
                    v3[st][:].rearrange("p (h e) -> p h e", h=HPC)[:, :, DH:65], 1.0)
            return emit

        def qk_unit(j, t, nb):
            def emit():
                ps = pst.tile([128, 512], F32, tag="st", name="psqk")
                for dc in range(NDC):
                    nc.tensor.matmul(
                        ps[:], wqk[(j, t)][:, dc * 128:(dc + 1) * 128],
                        xt[dc][:, nb * 512:(nb + 1) * 512],
                        start=(dc == 0), stop=(dc == NDC - 1),
                    )
                mt = t * NP + j
                dst = qt[j] if t == 0 else kt[j]
                nc.vector.tensor_scalar_add(
                    dst[:, nb * 512:(nb + 1) * 512], ps[:], bqk[:, mt:mt + 1])
            return emit

        # V-proj for the first four s-tiles runs before the attention stream
        # (PV(h0,kc) consumes v3[kc] two chunks in); the rest are fillers.
        for st in range(4):
            v_unit(st)()

        fillers = {}
        for st in range(4, NS):
            fillers.setdefault(st - 4, []).append(v_unit(st))       # h0: V-proj
        u = 16
        for t in range(2):
            for nb in range(NQ):
                fillers.setdefault(u, []).append(qk_unit(1, t, nb))  # h1: pair1
                u += 1
        u = 32
        for t in range(2):
            for nb in range(NQ):
                fillers.setdefault(u, []).append(qk_unit(2, t, nb))  # h2-h3: pair2
                u += 4
        u = 64
        for t in range(2):
            for nb in range(NQ):
                fillers.setdefault(u, []).append(qk_unit(3, t, nb))  # h4-h5: pair3
                u += 4

        opsum = {}
        e_tiles = {}
        oproj_backlog = []

        def emit_scores(h, kc):
            j, po = h // 2, (h % 2) * 64
            c0 = kc * 128
            e_t = e_pool.tile([128, S], BF16, tag="e", name="e")
            e_tiles[(h, kc)] = e_t
            seg0 = c0
            while seg0 < S:
                segw = min(1024 - seg0 % 1024, S - seg0)
                st_ps = pst.tile([128, 1024], F32, tag="st", name="st")
                sb = seg0 % 1024
                p0 = seg0
                while p0 < seg0 + segw:
                    pw = min(512 - p0 % 512, seg0 + segw - p0)
                    nc.tensor.matmul(
                        st_ps[:, p0 - seg0 + sb:p0 - seg0 + sb + pw],
                        kt[j][po:po + 64, kc * 128:(kc + 1) * 128],
                        qt[j][po:po + 64, p0:p0 + pw],
                        start=True, stop=True,
                    )
                    p0 += pw
                nc.scalar.activation(
                    e_t[:, seg0:seg0 + segw], st_ps[:, sb:sb + segw],
                    AF.Exp, scale=0.125)
                seg0 += segw
            # causal mask on the diagonal 128x128 block (gpsimd: idle engine);
            # the diagonal PV piece is emitted last, so this stays off the PE
            # critical path.
            nc.gpsimd.tensor_mul(
                e_t[:, c0:c0 + 128], e_t[:, c0:c0 + 128], tri[:])

        def emit_norm(h, qb):
            # pair j complete for this q-window: normalize + (pair 3) out-proj
            j = h // 2
            win = slice(qb * 512, (qb + 1) * 512)
            rps = pso.tile([128, 512], F32, tag="o", name="rps")
            nc.tensor.matmul(
                rps[:], sel[:, j * 128:(j + 1) * 128], recip[:, win],
                start=True, stop=True,
            )
            nc.vector.tensor_mul(oo[j][:, win], oo[j][:, win], rps[:])
            if j == NP - 1:
                for mt in range(qb * 4, qb * 4 + 4):
                    for half in range(2):
                        oproj_backlog.append((mt, half))

        ob_tiles = {}

        def emit_oproj(mt, half):
            ps = pso.tile([128, 512], F32, tag="o", name="pz")
            for jj in range(NP):
                nc.tensor.matmul(
                    ps[:],
                    oo[jj][:, mt * 128:(mt + 1) * 128],
                    wot[:, jj * 1024 + half * 512:jj * 1024 + half * 512 + 512],
                    start=(jj == 0), stop=(jj == NP - 1),
                )
            if half == 0:
                ob_tiles[mt] = drain_pool.tile([128, 1024], BF16, tag="ob", name="ob")
            nc.vector.tensor_copy(ob_tiles[mt][:, half * 512:(half + 1) * 512], ps[:])
            eng = nc.scalar if (mt >= 8 and (mt + half) % 2 == 0) else nc.sync
            eng.dma_start(
                out_d[mt * 128:(mt + 1) * 128, half * 512:(half + 1) * 512],
                ob_tiles[mt][:, half * 512:(half + 1) * 512])

        def emit_pv(h, kc):
            j, po = h // 2, (h % 2) * 64
            qb0, c0 = kc // 4, kc * 128
            e_t = e_tiles.pop((h, kc))
            if kc == 0:
                opsum[h] = [pso.tile([65, 512], F32, tag="o", name="opsum")
                            for _ in range(NQ)]
            vsl = v3[kc][:, h * 65:(h + 1) * 65]
            # non-diagonal pieces first; diagonal (mask-gated) piece last
            diag_only = (c0 + 128 == (qb0 + 1) * 512)
            if not diag_only:
                nc.tensor.matmul(
                    opsum[h][qb0][:, c0 + 128 - qb0 * 512:512],
                    vsl, e_t[:, c0 + 128:(qb0 + 1) * 512],
                    start=(kc == 0), stop=False,
                )
            for qb in range(qb0 + 1, NQ):
                nc.tensor.matmul(
                    opsum[h][qb][:], vsl, e_t[:, qb * 512:(qb + 1) * 512],
                    start=(kc == 0), stop=(kc == 4 * qb + 3),
                )
            nc.tensor.matmul(
                opsum[h][qb0][:, c0 - qb0 * 512:c0 + 128 - qb0 * 512],
                vsl, e_t[:, c0:c0 + 128],
                start=False, stop=(kc == 4 * qb0 + 3),
            )
            if kc % 4 == 3:
                qb = qb0
                win = slice(qb * 512, (qb + 1) * 512)
                stg = drain_pool.tile([65, 512], BF16, tag="stg", name="stg")
                nc.vector.tensor_copy(stg[:], opsum[h][qb][:])
                with nc.allow_low_precision(reason="softmax denom recip bf16"):
                    nc.vector.reciprocal(stg[64:65, :], stg[64:65, :])
                dr = 32 * j + h % 2
                # the final drain gates the tail: route it through the idle
                # Activation HWDGE queue instead of the congested SP queue
                dma_eng = nc.scalar if (h == HPC - 1 and qb == NQ - 1) else nc.sync
                dma_eng.dma_start(oo[j][po:po + 64, win], stg[0:64, :])
                dma_eng.dma_start(recip[dr:dr + 1, win], stg[64:65, :])
                if h % 2 == 1:
                    emit_norm(h, qb)

        SKEW = 2
        stream = [(h, kc) for h in range(HPC) for kc in range(NS)]
        for i, (h, kc) in enumerate(stream):
            emit_scores(h, kc)
            if i >= SKEW:
                emit_pv(*stream[i - SKEW])
            for unit in fillers.get(i, []):
                unit()
            for _ in range(3):
                if oproj_backlog:
                    emit_oproj(*oproj_backlog.pop(0))
        for i in range(SKEW, 0, -1):
            emit_pv(*stream[-i])
        while oproj_backlog:
            emit_oproj(*oproj_backlog.pop(0))


def _build():
    nc = bacc.Bacc("TRN2", target_bir_lowering=False, debug=False)
    xt_d = nc.dram_tensor("xt_s", [128, NDC * S], BF16, kind="ExternalInput").ap()
    wqk_d = nc.dram_tensor("wqk", [128, 8 * 1024], BF16, kind="ExternalInput").ap()
    bqk_d = nc.dram_tensor("bqk", [128, 8], F32, kind="ExternalInput").ap()
    wv_d = nc.dram_tensor("wv", [128, NDC * 512], BF16, kind="ExternalInput").ap()
    bv_d = nc.dram_tensor("bvb", [128, HPC * DH], F32, kind="ExternalInput").ap()
    wo_d = nc.dram_tensor("wo", [128, NP * 1024], BF16, kind="ExternalInput").ap()
    sel_d = nc.dram_tensor("sel", [128, NP * 128], BF16, kind="ExternalInput").ap()
    tri_d = nc.dram_tensor("tri", [128, 128], BF16, kind="ExternalInput").ap()
    out_d = nc.dram_tensor("out_s", [S, D], BF16, kind="ExternalOutput").ap()
    io = (xt_d, wqk_d, bqk_d, wv_d, bv_d, wo_d, sel_d, tri_d, out_d)
    with tile.TileContext(nc) as tc:
        with ExitStack() as ctx:
            _emit(ctx, tc, io)
    nc.compile()
    return nc


_NC = None


def _get_nc():
    global _NC
    if _NC is None:
        _NC = _build()
    return _NC


def _host_inputs(x, w_qkv, b_qkv, w_out):
    """Per-head-group shared weight arrays + per-core x, pre-arranged into
    SBUF layouts so every load is one large contiguous DMA."""
    import ml_dtypes
    maps = []
    hg_arrs = []
    for hg in range(2):
        hs = slice(hg * HPC, (hg + 1) * HPC)
        wq = np.asarray(w_qkv[:, 0, hs, :]).reshape(D, HPC * DH)
        wk = np.asarray(w_qkv[:, 1, hs, :]).reshape(D, HPC * DH)
        wqk_cat = np.concatenate([wq, wk], axis=1)  # [D, 2*HPC*DH], col mt*128+c
        # wqk chunked: [128, (mt, dc, 128)]
        wqk_arr = np.zeros((128, 8 * 1024), np.float32)
        for mt in range(8):
            for dc in range(NDC):
                wqk_arr[:, mt * 1024 + dc * 128:(mt * 1024 + dc * 128) + 128] = (
                    wqk_cat[dc * 128:(dc + 1) * 128, mt * 128:(mt + 1) * 128])
        bq = np.asarray(b_qkv[0, hs, :]).reshape(HPC * DH)
        bk = np.asarray(b_qkv[1, hs, :]).reshape(HPC * DH)
        bqk = np.zeros((128, 8), np.float32)
        for mt in range(8):
            t, j = mt // NP, mt % NP
            src = bq if t == 0 else bk
            bqk[:, mt] = src[j * 128:(j + 1) * 128]
        wv = np.asarray(w_qkv[:, 2, hs, :]).reshape(D, HPC * DH)
        # wv chunked: [128, (dc, 512)]
        wv_arr = np.zeros((128, NDC * 512), np.float32)
        for dc in range(NDC):
            wv_arr[:, dc * 512:(dc + 1) * 512] = wv[dc * 128:(dc + 1) * 128, :]
        bvb = np.broadcast_to(
            np.asarray(b_qkv[2, hs, :]).reshape(1, HPC * DH), (128, HPC * DH)
        ).astype(np.float32)
        wo = np.asarray(w_out[hs]).reshape(HPC * DH, D)
        # wo chunked: [128, (j, 1024)]
        wo_arr = np.zeros((128, NP * 1024), np.float32)
        for j in range(NP):
            wo_arr[:, j * 1024:(j + 1) * 1024] = wo[j * 128:(j + 1) * 128, :]
        selm = np.zeros((128, NP * 128), ml_dtypes.bfloat16)
        for j in range(NP):
            for p in range(128):
                selm[32 * j + p // 64, j * 128 + p] = 1.0
        trim = (np.arange(128)[None, :] >= np.arange(128)[:, None]).astype(
            ml_dtypes.bfloat16)
        hg_arrs.append(dict(
            wqk=np.ascontiguousarray(wqk_arr).astype(ml_dtypes.bfloat16),
            bqk=bqk,
            wv=np.ascontiguousarray(wv_arr).astype(ml_dtypes.bfloat16),
            bvb=bvb,
            wo=np.ascontiguousarray(wo_arr).astype(ml_dtypes.bfloat16),
            sel=selm, tri=trim))
    for c in range(8):
        b, hg = c % B, c // B
        m = dict(hg_arrs[hg])
        xb = np.asarray(x[b]).astype(ml_dtypes.bfloat16)  # [S, D]
        # xt chunked: [128, (dc, S)]: xt[p, dc*S + s] = x[s, dc*128+p]
        xtc = np.zeros((128, NDC * S), ml_dtypes.bfloat16)
        for dc in range(NDC):
            xtc[:, dc * S:(dc + 1) * S] = xb[:, dc * 128:(dc + 1) * 128].T
        m["xt_s"] = np.ascontiguousarray(xtc)
        maps.append(m)
    return maps


def _run(inputs, trace=False, tmpdir=None):
    nc = _get_nc()
    in_maps = _host_inputs(inputs["x"], inputs["w_qkv"], inputs["b_qkv"],
                           inputs["w_out"])
    res = bass_utils.run_bass_kernel_spmd(
        nc, in_maps, core_ids=list(range(8)), trace=trace, tmpdir=tmpdir)
    b_out = np.asarray(inputs["b_out"], dtype=np.float32)
    out = np.empty((B, S, D), np.float32)
    for b in range(B):
        out[b] = (res.results[b]["out_s"].astype(np.float32)
                  + res.results[b + B]["out_s"].astype(np.float32)
                  + b_out[None, :])
    return out, res


def kernel(**inputs) -> np.ndarray:
    out, _ = _run(inputs, trace=False)
    return out


# revision 33
# speedup vs baseline: 1.0087x; 1.0087x over previous
"""Causal attention block kernel for TRN2, 8 NeuronCores.

Sharding: 8 cores = 4 batches x 2 head-groups (8 heads each).
Each core computes, for its (batch, head-group):
  qkv = x @ w_qkv + b_qkv ; causal softmax attention ; partial out-proj.
Host sums the two head-group partials per batch and adds b_out.

Schedule (v2): all weights arrive via large host-prearranged contiguous
DMAs.  Pair-0 Q/K projection is dc-pipelined against the streamed X^T
chunk loads across 8 PSUM banks, so the PE starts ~2us in.  V-projection
and the remaining Q/K pairs are deferred filler units threaded through
the attention chunk stream, which runs with a 2-chunk scores->PV skew so
exp (scalar engine) latency stays off the PE critical path.  opsum banks
drain per q-block as soon as their accumulation stops (kc == 4qb+3);
normalization (reciprocal + selector broadcast) chases the drains per
(pair, q-block), and the output projection chases head 7's drains so
only the last q-block's projection trails the attention stream.
"""

import numpy as np
from contextlib import ExitStack

import concourse.bacc as bacc
import concourse.bass as bass
import concourse.mybir as mybir
import concourse.tile as tile
from concourse import bass_utils

F32 = mybir.dt.float32
BF16 = mybir.dt.bfloat16
AF = mybir.ActivationFunctionType

B, S, D, H, DH = 4, 2048, 1024, 16, 64
HPC = 8            # heads per core
NP = 4             # head pairs per core
NS = S // 128      # 16 s-tiles / k-chunks
NQ = S // 512      # 4 q-blocks
NDC = D // 128     # 8 d-chunks


def _emit(ctx: ExitStack, tc: tile.TileContext, io):
    nc = tc.nc
    xt_d, wqk_d, bqk_d, wv_d, bv_d, wo_d, sel_d, tri_d, out_d = io

    const = ctx.enter_context(tc.tile_pool(name="const", bufs=1))

    # ---- weight/constant loads (all large contiguous DMAs, issue order =
    # priority: pair0 QK weights, X^T chunks, V weights, consts, rest) ----
    wqk = {}
    xt = [const.tile([128, S], BF16, tag=f"xt{dc}", name=f"xt{dc}") for dc in range(NDC)]
    wvt = const.tile([128, NDC * 512], BF16, tag="wv", name="wvt")
    # first pair-0 weight half + first xt half land first so the dc=0
    # matmuls can issue ~2.5us in; halves keep each transfer short.
    for t in range(2):
        mt = t * NP
        wqk[(0, t)] = const.tile([128, 1024], BF16, tag=f"wq{0}{t}", name=f"wqk0{t}")
        nc.sync.dma_start(wqk[(0, t)][:, 0:512], wqk_d[:, mt * 1024:mt * 1024 + 512])
    nc.sync.dma_start(xt[0][:, 0:1024], xt_d[:, 0:1024])
    nc.sync.dma_start(xt[0][:, 1024:S], xt_d[:, 1024:S])
    bqk = const.tile([128, 8], F32, tag="bqk", name="bqk")
    bv = const.tile([128, HPC * DH], F32, tag="bv", name="bv")
    tri = const.tile([128, 128], BF16, tag="tri", name="tri")
    sel = const.tile([128, NP * 128], BF16, tag="sel", name="sel")
    for dc in range(1, NDC):
        nc.sync.dma_start(xt[dc][:], xt_d[:, dc * S:(dc + 1) * S])
        if dc == 1:
            # dc4-7 pair-0 weight halves: needed only from the dc=4 matmuls
            for t in range(2):
                mt = t * NP
                nc.sync.dma_start(
                    wqk[(0, t)][:, 512:1024],
                    wqk_d[:, mt * 1024 + 512:(mt + 1) * 1024])
            nc.sync.dma_start(bqk[:], bqk_d[:])
        elif dc == 2:
            nc.sync.dma_start(bv[:], bv_d[:])
        elif dc == 3:
            nc.sync.dma_start(tri[:], tri_d[:])
    # V weights right after the x^T stream (V-proj consumes them after the
    # pair-0 dc pipeline finishes); halves so dc0-3 can start first
    nc.sync.dma_start(wvt[:, 0:2048], wv_d[:, 0:2048])
    nc.sync.dma_start(wvt[:, 2048:4096], wv_d[:, 2048:4096])
    nc.sync.dma_start(sel[:], sel_d[:])
    for j in range(1, NP):
        for t in range(2):
            mt = t * NP + j
            wqk[(j, t)] = const.tile([128, 1024], BF16, tag=f"wq{j}{t}", name=f"wqk{j}{t}")
            nc.sync.dma_start(wqk[(j, t)][:], wqk_d[:, mt * 1024:(mt + 1) * 1024])
    wot = const.tile([128, NP * 1024], BF16, tag="wo", name="wot")
    nc.sync.dma_start(wot[:], wo_d[:])

    # ---- persistent attention tensors ----
    qkt_pool = ctx.enter_context(tc.tile_pool(name="qkt", bufs=10))
    v3_pool = ctx.enter_context(tc.tile_pool(name="v3", bufs=1))
    oo_pool = ctx.enter_context(tc.tile_pool(name="oo", bufs=1))
    dn_pool = ctx.enter_context(tc.tile_pool(name="dn", bufs=1))
    e_pool = ctx.enter_context(tc.tile_pool(name="epool", bufs=5))
    drain_pool = ctx.enter_context(tc.tile_pool(name="drain", bufs=4))

    qt = [qkt_pool.tile([128, S], BF16, tag="qkt", name="qkt") for j in range(NP)]
    kt = [qkt_pool.tile([128, S], BF16, tag="qkt", name="qkt") for j in range(NP)]
    v3 = [v3_pool.tile([128, HPC * 65], BF16, tag=f"v3_{st}", name=f"v3_{st}")
          for st in range(NS)]
    oo = [oo_pool.tile([128, S], BF16, tag=f"oo{j}", name=f"oo{j}") for j in range(NP)]
    # head h's reciprocal denominator row lives at partition 32*(h//2) + h%2
    # (reciprocal happens on the drain staging tile; the DMA ships it here)
    recip = dn_pool.tile([128, S], BF16, tag="recip", name="recip")
    # selector matmul reads all 8 recip rows before late pairs are written;
    # 0*garbage must stay 0, so clear once (NaN guard).
    nc.vector.memset(recip[:], 1.0)
    # 1-element dummy activation: pulls the ACT table load to t~1us so the
    # first real ScalarE ops (pair-0 bias adds) aren't serialized behind it
    nc.scalar.activation(recip[0:1, 0:1], recip[0:1, 0:1], AF.Exp)

    # ---- phase 1: pair-0 Q/K projection, dc-pipelined over 8 psum banks ----
    psA_ctx = tc.tile_pool(name="psA", bufs=8, space="PSUM")
    psA = psA_ctx.__enter__()
    if True:
        ps0 = [psA.tile([128, 512], F32, tag="qk0", name="psqk0") for _ in range(8)]
        for dc in range(NDC):
            for nb in range(NQ):
                for ti in range(2):
                    nc.tensor.matmul(
                        ps0[ti * 4 + nb][:],
                        wqk[(0, ti)][:, dc * 128:(dc + 1) * 128],
                        xt[dc][:, nb * 512:(nb + 1) * 512],
                        start=(dc == 0), stop=(dc == NDC - 1),
                    )
        # bias adds split across DVE and ScalarE, emitted in BANK order: the
        # attention pools reuse these psum banks (low banks first), so the
        # low-bank biases must complete first or the first V-proj/score
        # matmuls stall on the pool-transition WAR
        for bank in range(8):
            ti, nb = bank // 4, bank % 4
            mt = ti * NP
            dst = qt[0] if ti == 0 else kt[0]
            if bank % 2 == 0:
                nc.vector.tensor_scalar_add(
                    dst[:, nb * 512:(nb + 1) * 512], ps0[bank][:],
                    bqk[:, mt:mt + 1])
            else:
                nc.scalar.activation(
                    dst[:, nb * 512:(nb + 1) * 512], ps0[bank][:],
                    AF.Identity, bias=bqk[:, mt:mt + 1])

    # ---- phase 2: attention stream with filler units ----
    class _PSUM:
        pass
    if True:
        def v_unit(st, pool=None):
            def emit():
                ps = (pool or pst).tile([128, 512], F32,
                                        tag="st" if pool is None else "qk0",
                                        name="psv")
                for dc in range(NDC):
                    nc.tensor.matmul(
                        ps[:], xt[dc][:, st * 128:(st + 1) * 128],
                        wvt[:, dc * 512:(dc + 1) * 512],
                        start=(dc == 0), stop=(dc == NDC - 1),
                    )
                v_view = v3[st][:].rearrange("p (h e) -> p h e", h=HPC)[:, :, 0:DH]
                nc.vector.tensor_add(
                    v_view,
                    ps[:].rearrange("p (h e) -> p h e", h=HPC),
                    bv[:].rearrange("p (h e) -> p h e", h=HPC),
                )
                nc.vector.memset(
                    v3[st][:].rearrange("p (h e) -> p h e", h=HPC)[:, :, DH:65], 1.0)
            return emit

        def qk_unit(j, t, nb):
            def emit():
                ps = pst.tile([128, 512], F32, tag="st", name="psqk")
                for dc in range(NDC):
                    nc.tensor.matmul(
                        ps[:], wqk[(j, t)][:, dc * 128:(dc + 1) * 128],
                        xt[dc][:, nb * 512:(nb + 1) * 512],
                        start=(dc == 0), stop=(dc == NDC - 1),
                    )
                mt = t * NP + j
                dst = qt[j] if t == 0 else kt[j]
                nc.vector.tensor_scalar_add(
                    dst[:, nb * 512:(nb + 1) * 512], ps[:], bqk[:, mt:mt + 1])
            return emit

        # V-proj for the first four s-tiles runs before the attention stream
        # (PV(h0,kc) consumes v3[kc] two chunks in), inside the phase-1 psum
        # pool (its 8-slot ring avoids the pool-transition WAR on the bias
        # adds); the rest are fillers in the attention pools.
        for st in range(4):
            v_unit(st, pool=psA)()
        psA_ctx.__exit__(None, None, None)
        pst = ctx.enter_context(tc.tile_pool(name="pst", bufs=2, space="PSUM"))
        pso = ctx.enter_context(tc.tile_pool(name="pso", bufs=4, space="PSUM"))

        fillers = {}
        for st in range(4, NS):
            fillers.setdefault(st - 4, []).append(v_unit(st))       # h0: V-proj
        u = 16
        for t in range(2):
            for nb in range(NQ):
                fillers.setdefault(u, []).append(qk_unit(1, t, nb))  # h1: pair1
                u += 1
        # pair2/pair3 units weighted toward head-start chunks, where PV(h,0/1)
        # otherwise starves waiting on the new head's big exp segments
        p2_slots = [32, 33, 36, 40, 48, 49, 52, 56]
        p3_slots = [64, 65, 68, 72, 80, 81, 84, 88]
        units2 = [qk_unit(2, t, nb) for t in range(2) for nb in range(NQ)]
        units3 = [qk_unit(3, t, nb) for t in range(2) for nb in range(NQ)]
        for slot, unit in zip(p2_slots, units2):
            fillers.setdefault(slot, []).append(unit)
        for slot, unit in zip(p3_slots, units3):
            fillers.setdefault(slot, []).append(unit)

        opsum = {}
        e_tiles = {}
        oproj_backlog = []

        def emit_scores(h, kc):
            j, po = h // 2, (h % 2) * 64
            c0 = kc * 128
            e_t = e_pool.tile([128, S], BF16, tag="e", name="e")
            e_tiles[(h, kc)] = e_t
            seg0 = c0
            while seg0 < S:
                segw = min(1024 - seg0 % 1024, S - seg0)
                st_ps = pst.tile([128, 1024], F32, tag="st", name="st")
                sb = seg0 % 1024
                p0 = seg0
                while p0 < seg0 + segw:
                    pw = min(512 - p0 % 512, seg0 + segw - p0)
                    nc.tensor.matmul(
                        st_ps[:, p0 - seg0 + sb:p0 - seg0 + sb + pw],
                        kt[j][po:po + 64, kc * 128:(kc + 1) * 128],
                        qt[j][po:po + 64, p0:p0 + pw],
                        start=True, stop=True,
                    )
                    p0 += pw
                nc.scalar.activation(
                    e_t[:, seg0:seg0 + segw], st_ps[:, sb:sb + segw],
                    AF.Exp, scale=0.125)
                seg0 += segw
            # causal mask on the diagonal 128x128 block (gpsimd: idle engine);
            # the diagonal PV piece is emitted last, so this stays off the PE
            # critical path.
            nc.gpsimd.tensor_mul(
                e_t[:, c0:c0 + 128], e_t[:, c0:c0 + 128], tri[:])

        def emit_norm(h, qb):
            # pair j complete for this q-window: normalize + (pair 3) out-proj
            j = h // 2
            win = slice(qb * 512, (qb + 1) * 512)
            rps = pso.tile([128, 512], F32, tag="o", name="rps")
            nc.tensor.matmul(
                rps[:], sel[:, j * 128:(j + 1) * 128], recip[:, win],
                start=True, stop=True,
            )
            nc.vector.tensor_mul(oo[j][:, win], oo[j][:, win], rps[:])
            if j == NP - 1:
                for mt in range(qb * 4, qb * 4 + 4):
                    for half in range(2):
                        oproj_backlog.append((mt, half))

        ob_tiles = {}

        def emit_oproj(mt, half):
            ps = pso.tile([128, 512], F32, tag="o", name="pz")
            for jj in range(NP):
                nc.tensor.matmul(
                    ps[:],
                    oo[jj][:, mt * 128:(mt + 1) * 128],
                    wot[:, jj * 1024 + half * 512:jj * 1024 + half * 512 + 512],
                    start=(jj == 0), stop=(jj == NP - 1),
                )
            if half == 0:
                ob_tiles[mt] = drain_pool.tile([128, 1024], BF16, tag="ob", name="ob")
            nc.vector.tensor_copy(ob_tiles[mt][:, half * 512:(half + 1) * 512], ps[:])
            eng = nc.scalar if (mt >= 8 and (mt + half) % 2 == 0) else nc.sync
            eng.dma_start(
                out_d[mt * 128:(mt + 1) * 128, half * 512:(half + 1) * 512],
                ob_tiles[mt][:, half * 512:(half + 1) * 512])

        def emit_pv(h, kc):
            j, po = h // 2, (h % 2) * 64
            qb0, c0 = kc // 4, kc * 128
            e_t = e_tiles.pop((h, kc))
            if kc == 0:
                opsum[h] = [pso.tile([65, 512], F32, tag="o", name="opsum")
                            for _ in range(NQ)]
            vsl = v3[kc][:, h * 65:(h + 1) * 65]
            # non-diagonal pieces first; diagonal (mask-gated) piece last
            diag_only = (c0 + 128 == (qb0 + 1) * 512)
            if not diag_only:
                nc.tensor.matmul(
                    opsum[h][qb0][:, c0 + 128 - qb0 * 512:512],
                    vsl, e_t[:, c0 + 128:(qb0 + 1) * 512],
                    start=(kc == 0), stop=False,
                )
            for qb in range(qb0 + 1, NQ):
                nc.tensor.matmul(
                    opsum[h][qb][:], vsl, e_t[:, qb * 512:(qb + 1) * 512],
                    start=(kc == 0), stop=(kc == 4 * qb + 3),
                )
            nc.tensor.matmul(
                opsum[h][qb0][:, c0 - qb0 * 512:c0 + 128 - qb0 * 512],
                vsl, e_t[:, c0:c0 + 128],
                start=False, stop=(kc == 4 * qb0 + 3),
            )
            if kc % 4 == 3:
                qb = qb0
                win = slice(qb * 512, (qb + 1) * 512)
                stg = drain_pool.tile([65, 512], BF16, tag="stg", name="stg")
                nc.vector.tensor_copy(stg[:], opsum[h][qb][:])
                with nc.allow_low_precision(reason="softmax denom recip bf16"):
                    nc.vector.reciprocal(stg[64:65, :], stg[64:65, :])
                dr = 32 * j + h % 2
                # the final drain gates the tail: route it through the idle
                # Activation HWDGE queue instead of the congested SP queue
                dma_eng = nc.scalar if (h == HPC - 1 and qb == NQ - 1) else nc.sync
                dma_eng.dma_start(oo[j][po:po + 64, win], stg[0:64, :])
                dma_eng.dma_start(recip[dr:dr + 1, win], stg[64:65, :])
                if h % 2 == 1:
                    emit_norm(h, qb)

        SKEW = 2
        stream = [(h, kc) for h in range(HPC) for kc in range(NS)]
        for i, (h, kc) in enumerate(stream):
            emit_scores(h, kc)
            if i >= SKEW:
                emit_pv(*stream[i - SKEW])
            for unit in fillers.get(i, []):
                unit()
            for _ in range(3):
                if oproj_backlog:
                    emit_oproj(*oproj_backlog.pop(0))
        for i in range(SKEW, 0, -1):
            emit_pv(*stream[-i])
        while oproj_backlog:
            emit_oproj(*oproj_backlog.pop(0))


def _build():
    nc = bacc.Bacc("TRN2", target_bir_lowering=False, debug=False)
    xt_d = nc.dram_tensor("xt_s", [128, NDC * S], BF16, kind="ExternalInput").ap()
    wqk_d = nc.dram_tensor("wqk", [128, 8 * 1024], BF16, kind="ExternalInput").ap()
    bqk_d = nc.dram_tensor("bqk", [128, 8], F32, kind="ExternalInput").ap()
    wv_d = nc.dram_tensor("wv", [128, NDC * 512], BF16, kind="ExternalInput").ap()
    bv_d = nc.dram_tensor("bvb", [128, HPC * DH], F32, kind="ExternalInput").ap()
    wo_d = nc.dram_tensor("wo", [128, NP * 1024], BF16, kind="ExternalInput").ap()
    sel_d = nc.dram_tensor("sel", [128, NP * 128], BF16, kind="ExternalInput").ap()
    tri_d = nc.dram_tensor("tri", [128, 128], BF16, kind="ExternalInput").ap()
    out_d = nc.dram_tensor("out_s", [S, D], BF16, kind="ExternalOutput").ap()
    io = (xt_d, wqk_d, bqk_d, wv_d, bv_d, wo_d, sel_d, tri_d, out_d)
    with tile.TileContext(nc) as tc:
        with ExitStack() as ctx:
            _emit(ctx, tc, io)
    nc.compile()
    return nc


_NC = None


def _get_nc():
    global _NC
    if _NC is None:
        _NC = _build()
    return _NC


def _host_inputs(x, w_qkv, b_qkv, w_out):
    """Per-head-group shared weight arrays + per-core x, pre-arranged into
    SBUF layouts so every load is one large contiguous DMA."""
    import ml_dtypes
    maps = []
    hg_arrs = []
    for hg in range(2):
        hs = slice(hg * HPC, (hg + 1) * HPC)
        wq = np.asarray(w_qkv[:, 0, hs, :]).reshape(D, HPC * DH)
        wk = np.asarray(w_qkv[:, 1, hs, :]).reshape(D, HPC * DH)
        wqk_cat = np.concatenate([wq, wk], axis=1)  # [D, 2*HPC*DH], col mt*128+c
        # wqk chunked: [128, (mt, dc, 128)]
        wqk_arr = np.zeros((128, 8 * 1024), np.float32)
        for mt in range(8):
            for dc in range(NDC):
                wqk_arr[:, mt * 1024 + dc * 128:(mt * 1024 + dc * 128) + 128] = (
                    wqk_cat[dc * 128:(dc + 1) * 128, mt * 128:(mt + 1) * 128])
        bq = np.asarray(b_qkv[0, hs, :]).reshape(HPC * DH)
        bk = np.asarray(b_qkv[1, hs, :]).reshape(HPC * DH)
        bqk = np.zeros((128, 8), np.float32)
        for mt in range(8):
            t, j = mt // NP, mt % NP
            src = bq if t == 0 else bk
            bqk[:, mt] = src[j * 128:(j + 1) * 128]
        wv = np.asarray(w_qkv[:, 2, hs, :]).reshape(D, HPC * DH)
        # wv chunked: [128, (dc, 512)]
        wv_arr = np.zeros((128, NDC * 512), np.float32)
        for dc in range(NDC):
            wv_arr[:, dc * 512:(dc + 1) * 512] = wv[dc * 128:(dc + 1) * 128, :]
        bvb = np.broadcast_to(
            np.asarray(b_qkv[2, hs, :]).reshape(1, HPC * DH), (128, HPC * DH)
        ).astype(np.float32)
        wo = np.asarray(w_out[hs]).reshape(HPC * DH, D)
        # wo chunked: [128, (j, 1024)]
        wo_arr = np.zeros((128, NP * 1024), np.float32)
        for j in range(NP):
            wo_arr[:, j * 1024:(j + 1) * 1024] = wo[j * 128:(j + 1) * 128, :]
        selm = np.zeros((128, NP * 128), ml_dtypes.bfloat16)
        for j in range(NP):
            for p in range(128):
                selm[32 * j + p // 64, j * 128 + p] = 1.0
        trim = (np.arange(128)[None, :] >= np.arange(128)[:, None]).astype(
            ml_dtypes.bfloat16)
        hg_arrs.append(dict(
            wqk=np.ascontiguousarray(wqk_arr).astype(ml_dtypes.bfloat16),
            bqk=bqk,
            wv=np.ascontiguousarray(wv_arr).astype(ml_dtypes.bfloat16),
            bvb=bvb,
            wo=np.ascontiguousarray(wo_arr).astype(ml_dtypes.bfloat16),
            sel=selm, tri=trim))
    for c in range(8):
        b, hg = c % B, c // B
        m = dict(hg_arrs[hg])
        xb = np.asarray(x[b]).astype(ml_dtypes.bfloat16)  # [S, D]
        # xt chunked: [128, (dc, S)]: xt[p, dc*S + s] = x[s, dc*128+p]
        xtc = np.zeros((128, NDC * S), ml_dtypes.bfloat16)
        for dc in range(NDC):
            xtc[:, dc * S:(dc + 1) * S] = xb[:, dc * 128:(dc + 1) * 128].T
        m["xt_s"] = np.ascontiguousarray(xtc)
        maps.append(m)
    return maps


def _run(inputs, trace=False, tmpdir=None):
    nc = _get_nc()
    in_maps = _host_inputs(inputs["x"], inputs["w_qkv"], inputs["b_qkv"],
                           inputs["w_out"])
    res = bass_utils.run_bass_kernel_spmd(
        nc, in_maps, core_ids=list(range(8)), trace=trace, tmpdir=tmpdir)
    b_out = np.asarray(inputs["b_out"], dtype=np.float32)
    out = np.empty((B, S, D), np.float32)
    for b in range(B):
        out[b] = (res.results[b]["out_s"].astype(np.float32)
                  + res.results[b + B]["out_s"].astype(np.float32)
                  + b_out[None, :])
    return out, res


def kernel(**inputs) -> np.ndarray:
    out, _ = _run(inputs, trace=False)
    return out


# revision 37
# speedup vs baseline: 1.0099x; 1.0012x over previous
"""Causal attention block kernel for TRN2, 8 NeuronCores.

Sharding: 8 cores = 4 batches x 2 head-groups (8 heads each).
Each core computes, for its (batch, head-group):
  qkv = x @ w_qkv + b_qkv ; causal softmax attention ; partial out-proj.
Host sums the two head-group partials per batch and adds b_out.

Schedule (v2): all weights arrive via large host-prearranged contiguous
DMAs.  Pair-0 Q/K projection is dc-pipelined against the streamed X^T
chunk loads across 8 PSUM banks, so the PE starts ~2us in.  V-projection
and the remaining Q/K pairs are deferred filler units threaded through
the attention chunk stream, which runs with a 2-chunk scores->PV skew so
exp (scalar engine) latency stays off the PE critical path.  opsum banks
drain per q-block as soon as their accumulation stops (kc == 4qb+3);
normalization (reciprocal + selector broadcast) chases the drains per
(pair, q-block), and the output projection chases head 7's drains so
only the last q-block's projection trails the attention stream.
"""

import numpy as np
from contextlib import ExitStack

import concourse.bacc as bacc
import concourse.bass as bass
import concourse.mybir as mybir
import concourse.tile as tile
from concourse import bass_utils

F32 = mybir.dt.float32
BF16 = mybir.dt.bfloat16
AF = mybir.ActivationFunctionType

B, S, D, H, DH = 4, 2048, 1024, 16, 64
HPC = 8            # heads per core
NP = 4             # head pairs per core
NS = S // 128      # 16 s-tiles / k-chunks
NQ = S // 512      # 4 q-blocks
NDC = D // 128     # 8 d-chunks


def _emit(ctx: ExitStack, tc: tile.TileContext, io):
    nc = tc.nc
    xt_d, wqk_d, bqk_d, wv_d, bv_d, wo_d, sel_d, tri_d, out_d = io

    const = ctx.enter_context(tc.tile_pool(name="const", bufs=1))

    # ---- weight/constant loads (all large contiguous DMAs, issue order =
    # priority: pair0 QK weights, X^T chunks, V weights, consts, rest) ----
    wqk = {}
    xt = [const.tile([128, S], BF16, tag=f"xt{dc}", name=f"xt{dc}") for dc in range(NDC)]
    wvt = const.tile([128, NDC * 512], BF16, tag="wv", name="wvt")
    # first pair-0 weight half + first xt half land first so the dc=0
    # matmuls can issue ~2.5us in; halves keep each transfer short.
    for t in range(2):
        mt = t * NP
        wqk[(0, t)] = const.tile([128, 1024], BF16, tag=f"wq{0}{t}", name=f"wqk0{t}")
        nc.sync.dma_start(wqk[(0, t)][:, 0:512], wqk_d[:, mt * 1024:mt * 1024 + 512])
    nc.sync.dma_start(xt[0][:, 0:1024], xt_d[:, 0:1024])
    nc.sync.dma_start(xt[0][:, 1024:S], xt_d[:, 1024:S])
    bqk = const.tile([128, 8], F32, tag="bqk", name="bqk")
    bv = const.tile([128, HPC * DH], F32, tag="bv", name="bv")
    tri = const.tile([128, 128], BF16, tag="tri", name="tri")
    sel = const.tile([128, NP * 128], BF16, tag="sel", name="sel")
    for dc in range(1, NDC):
        nc.sync.dma_start(xt[dc][:], xt_d[:, dc * S:(dc + 1) * S])
        if dc == 1:
            # dc4-7 pair-0 weight halves: needed only from the dc=4 matmuls
            for t in range(2):
                mt = t * NP
                nc.sync.dma_start(
                    wqk[(0, t)][:, 512:1024],
                    wqk_d[:, mt * 1024 + 512:(mt + 1) * 1024])
            nc.sync.dma_start(bqk[:], bqk_d[:])
        elif dc == 2:
            nc.sync.dma_start(bv[:], bv_d[:])
        elif dc == 3:
            nc.sync.dma_start(tri[:], tri_d[:])
    # V weights right after the x^T stream (V-proj consumes them after the
    # pair-0 dc pipeline finishes); halves so dc0-3 can start first
    nc.sync.dma_start(wvt[:, 0:2048], wv_d[:, 0:2048])
    nc.sync.dma_start(wvt[:, 2048:4096], wv_d[:, 2048:4096])
    nc.sync.dma_start(sel[:], sel_d[:])
    for j in range(1, NP):
        for t in range(2):
            mt = t * NP + j
            wqk[(j, t)] = const.tile([128, 1024], BF16, tag=f"wq{j}{t}", name=f"wqk{j}{t}")
            nc.sync.dma_start(wqk[(j, t)][:], wqk_d[:, mt * 1024:(mt + 1) * 1024])
    wot = const.tile([128, NP * 1024], BF16, tag="wo", name="wot")
    nc.sync.dma_start(wot[:], wo_d[:])

    # ---- persistent attention tensors ----
    qkt_pool = ctx.enter_context(tc.tile_pool(name="qkt", bufs=10))
    v3_pool = ctx.enter_context(tc.tile_pool(name="v3", bufs=1))
    oo_pool = ctx.enter_context(tc.tile_pool(name="oo", bufs=1))
    dn_pool = ctx.enter_context(tc.tile_pool(name="dn", bufs=1))
    e_pool = ctx.enter_context(tc.tile_pool(name="epool", bufs=5))
    drain_pool = ctx.enter_context(tc.tile_pool(name="drain", bufs=4))

    qt = [qkt_pool.tile([128, S], BF16, tag="qkt", name="qkt") for j in range(NP)]
    kt = [qkt_pool.tile([128, S], BF16, tag="qkt", name="qkt") for j in range(NP)]
    v3 = [v3_pool.tile([128, HPC * 65], BF16, tag=f"v3_{st}", name=f"v3_{st}")
          for st in range(NS)]
    oo = [oo_pool.tile([128, S], BF16, tag=f"oo{j}", name=f"oo{j}") for j in range(NP)]
    # head h's reciprocal denominator row lives at partition 32*(h//2) + h%2
    # (reciprocal happens on the drain staging tile; the DMA ships it here)
    recip = dn_pool.tile([128, S], BF16, tag="recip", name="recip")
    # selector matmul reads all 8 recip rows before late pairs are written;
    # 0*garbage must stay 0, so clear once (NaN guard).
    nc.vector.memset(recip[:], 1.0)
    # 1-element dummy activation: pulls the ACT table load to t~1us so the
    # first real ScalarE ops (pair-0 bias adds) aren't serialized behind it
    nc.scalar.activation(recip[0:1, 0:1], recip[0:1, 0:1], AF.Exp)

    # ---- phase 1: pair-0 Q/K projection, dc-pipelined over 8 psum banks ----
    psA_ctx = tc.tile_pool(name="psA", bufs=8, space="PSUM")
    psA = psA_ctx.__enter__()
    if True:
        ps0 = [psA.tile([128, 512], F32, tag="qk0", name="psqk0") for _ in range(8)]
        for dc in range(NDC):
            for nb in range(NQ):
                for ti in range(2):
                    nc.tensor.matmul(
                        ps0[ti * 4 + nb][:],
                        wqk[(0, ti)][:, dc * 128:(dc + 1) * 128],
                        xt[dc][:, nb * 512:(nb + 1) * 512],
                        start=(dc == 0), stop=(dc == NDC - 1),
                    )
        # bias adds split across DVE and ScalarE, emitted in BANK order: the
        # attention pools reuse these psum banks (low banks first), so the
        # low-bank biases must complete first or the first V-proj/score
        # matmuls stall on the pool-transition WAR
        for bank in range(8):
            ti, nb = bank // 4, bank % 4
            mt = ti * NP
            dst = qt[0] if ti == 0 else kt[0]
            if bank % 2 == 0:
                nc.vector.tensor_scalar_add(
                    dst[:, nb * 512:(nb + 1) * 512], ps0[bank][:],
                    bqk[:, mt:mt + 1])
            else:
                nc.scalar.activation(
                    dst[:, nb * 512:(nb + 1) * 512], ps0[bank][:],
                    AF.Identity, bias=bqk[:, mt:mt + 1])

    # ---- phase 2: attention stream with filler units ----
    class _PSUM:
        pass
    if True:
        def v_unit(st, pool=None):
            def emit():
                ps = (pool or pst).tile([128, 512], F32,
                                        tag="st" if pool is None else "qk0",
                                        name="psv")
                for dc in range(NDC):
                    nc.tensor.matmul(
                        ps[:], xt[dc][:, st * 128:(st + 1) * 128],
                        wvt[:, dc * 512:(dc + 1) * 512],
                        start=(dc == 0), stop=(dc == NDC - 1),
                    )
                v_view = v3[st][:].rearrange("p (h e) -> p h e", h=HPC)[:, :, 0:DH]
                nc.vector.tensor_add(
                    v_view,
                    ps[:].rearrange("p (h e) -> p h e", h=HPC),
                    bv[:].rearrange("p (h e) -> p h e", h=HPC),
                )
                nc.vector.memset(
                    v3[st][:].rearrange("p (h e) -> p h e", h=HPC)[:, :, DH:65], 1.0)
            return emit

        def qk_unit(j, t, nb):
            def emit():
                ps = pst.tile([128, 512], F32, tag="st", name="psqk")
                for dc in range(NDC):
                    nc.tensor.matmul(
                        ps[:], wqk[(j, t)][:, dc * 128:(dc + 1) * 128],
                        xt[dc][:, nb * 512:(nb + 1) * 512],
                        start=(dc == 0), stop=(dc == NDC - 1),
                    )
                mt = t * NP + j
                dst = qt[j] if t == 0 else kt[j]
                nc.vector.tensor_scalar_add(
                    dst[:, nb * 512:(nb + 1) * 512], ps[:], bqk[:, mt:mt + 1])
            return emit

        # V-proj for the first four s-tiles runs before the attention stream
        # (PV(h0,kc) consumes v3[kc] two chunks in), inside the phase-1 psum
        # pool (its 8-slot ring avoids the pool-transition WAR on the bias
        # adds); the rest are fillers in the attention pools.
        for st in range(6):
            v_unit(st, pool=psA)()
        psA_ctx.__exit__(None, None, None)
        pst = ctx.enter_context(tc.tile_pool(name="pst", bufs=2, space="PSUM"))
        pso = ctx.enter_context(tc.tile_pool(name="pso", bufs=4, space="PSUM"))

        fillers = {}
        for st in range(6, NS):
            fillers.setdefault(st - 6, []).append(v_unit(st))       # h0: V-proj
        u = 16
        for t in range(2):
            for nb in range(NQ):
                fillers.setdefault(u, []).append(qk_unit(1, t, nb))  # h1: pair1
                u += 1
        # pair2/pair3 units weighted toward head-start chunks, where PV(h,0/1)
        # otherwise starves waiting on the new head's big exp segments
        p2_slots = [32, 33, 36, 40, 48, 49, 52, 56]
        p3_slots = [64, 65, 68, 72, 80, 81, 84, 88]
        units2 = [qk_unit(2, t, nb) for t in range(2) for nb in range(NQ)]
        units3 = [qk_unit(3, t, nb) for t in range(2) for nb in range(NQ)]
        for slot, unit in zip(p2_slots, units2):
            fillers.setdefault(slot, []).append(unit)
        for slot, unit in zip(p3_slots, units3):
            fillers.setdefault(slot, []).append(unit)

        opsum = {}
        e_tiles = {}
        oproj_backlog = []

        def emit_scores(h, kc):
            j, po = h // 2, (h % 2) * 64
            c0 = kc * 128
            e_t = e_pool.tile([128, S], BF16, tag="e", name="e")
            e_tiles[(h, kc)] = e_t
            seg0 = c0
            while seg0 < S:
                segw = min(1024 - seg0 % 1024, S - seg0)
                st_ps = pst.tile([128, 1024], F32, tag="st", name="st")
                sb = seg0 % 1024
                p0 = seg0
                while p0 < seg0 + segw:
                    pw = min(512 - p0 % 512, seg0 + segw - p0)
                    nc.tensor.matmul(
                        st_ps[:, p0 - seg0 + sb:p0 - seg0 + sb + pw],
                        kt[j][po:po + 64, kc * 128:(kc + 1) * 128],
                        qt[j][po:po + 64, p0:p0 + pw],
                        start=True, stop=True,
                    )
                    p0 += pw
                nc.scalar.activation(
                    e_t[:, seg0:seg0 + segw], st_ps[:, sb:sb + segw],
                    AF.Exp, scale=0.125)
                seg0 += segw
            # causal mask on the diagonal 128x128 block (gpsimd: idle engine);
            # the diagonal PV piece is emitted last, so this stays off the PE
            # critical path.
            nc.gpsimd.tensor_mul(
                e_t[:, c0:c0 + 128], e_t[:, c0:c0 + 128], tri[:])

        def emit_norm(h, qb):
            # pair j complete for this q-window: normalize + (pair 3) out-proj
            j = h // 2
            win = slice(qb * 512, (qb + 1) * 512)
            rps = pso.tile([128, 512], F32, tag="o", name="rps")
            nc.tensor.matmul(
                rps[:], sel[:, j * 128:(j + 1) * 128], recip[:, win],
                start=True, stop=True,
            )
            nc.vector.tensor_mul(oo[j][:, win], oo[j][:, win], rps[:])
            if j == NP - 1:
                for mt in range(qb * 4, qb * 4 + 4):
                    for half in range(2):
                        oproj_backlog.append((mt, half))

        ob_tiles = {}

        def emit_oproj(mt, half):
            ps = pso.tile([128, 512], F32, tag="o", name="pz")
            for jj in range(NP):
                nc.tensor.matmul(
                    ps[:],
                    oo[jj][:, mt * 128:(mt + 1) * 128],
                    wot[:, jj * 1024 + half * 512:jj * 1024 + half * 512 + 512],
                    start=(jj == 0), stop=(jj == NP - 1),
                )
            if half == 0:
                ob_tiles[mt] = drain_pool.tile([128, 1024], BF16, tag="ob", name="ob")
            nc.vector.tensor_copy(ob_tiles[mt][:, half * 512:(half + 1) * 512], ps[:])
            eng = nc.scalar if (mt >= 8 and (mt + half) % 2 == 0) else nc.sync
            eng.dma_start(
                out_d[mt * 128:(mt + 1) * 128, half * 512:(half + 1) * 512],
                ob_tiles[mt][:, half * 512:(half + 1) * 512])

        def emit_pv(h, kc):
            j, po = h // 2, (h % 2) * 64
            qb0, c0 = kc // 4, kc * 128
            e_t = e_tiles.pop((h, kc))
            if kc == 0:
                opsum[h] = [pso.tile([65, 512], F32, tag="o", name="opsum")
                            for _ in range(NQ)]
            vsl = v3[kc][:, h * 65:(h + 1) * 65]
            # non-diagonal pieces first; diagonal (mask-gated) piece last
            diag_only = (c0 + 128 == (qb0 + 1) * 512)
            if not diag_only:
                nc.tensor.matmul(
                    opsum[h][qb0][:, c0 + 128 - qb0 * 512:512],
                    vsl, e_t[:, c0 + 128:(qb0 + 1) * 512],
                    start=(kc == 0), stop=False,
                )
            for qb in range(qb0 + 1, NQ):
                nc.tensor.matmul(
                    opsum[h][qb][:], vsl, e_t[:, qb * 512:(qb + 1) * 512],
                    start=(kc == 0), stop=(kc == 4 * qb + 3),
                )
            nc.tensor.matmul(
                opsum[h][qb0][:, c0 - qb0 * 512:c0 + 128 - qb0 * 512],
                vsl, e_t[:, c0:c0 + 128],
                start=False, stop=(kc == 4 * qb0 + 3),
            )
            if kc % 4 == 3:
                qb = qb0
                win = slice(qb * 512, (qb + 1) * 512)
                stg = drain_pool.tile([65, 512], BF16, tag="stg", name="stg")
                nc.vector.tensor_copy(stg[:], opsum[h][qb][:])
                with nc.allow_low_precision(reason="softmax denom recip bf16"):
                    nc.vector.reciprocal(stg[64:65, :], stg[64:65, :])
                dr = 32 * j + h % 2
                # the final drain gates the tail: route it through the idle
                # Activation HWDGE queue instead of the congested SP queue
                dma_eng = nc.scalar if (h == HPC - 1 and qb == NQ - 1) else nc.sync
                dma_eng.dma_start(oo[j][po:po + 64, win], stg[0:64, :])
                dma_eng.dma_start(recip[dr:dr + 1, win], stg[64:65, :])
                if h % 2 == 1:
                    emit_norm(h, qb)

        SKEW = 2
        stream = [(h, kc) for h in range(HPC) for kc in range(NS)]
        for i, (h, kc) in enumerate(stream):
            emit_scores(h, kc)
            if i >= SKEW:
                emit_pv(*stream[i - SKEW])
            for unit in fillers.get(i, []):
                unit()
            for _ in range(3):
                if oproj_backlog:
                    emit_oproj(*oproj_backlog.pop(0))
        for i in range(SKEW, 0, -1):
            emit_pv(*stream[-i])
        while oproj_backlog:
            emit_oproj(*oproj_backlog.pop(0))


def _build():
    nc = bacc.Bacc("TRN2", target_bir_lowering=False, debug=False)
    xt_d = nc.dram_tensor("xt_s", [128, NDC * S], BF16, kind="ExternalInput").ap()
    wqk_d = nc.dram_tensor("wqk", [128, 8 * 1024], BF16, kind="ExternalInput").ap()
    bqk_d = nc.dram_tensor("bqk", [128, 8], F32, kind="ExternalInput").ap()
    wv_d = nc.dram_tensor("wv", [128, NDC * 512], BF16, kind="ExternalInput").ap()
    bv_d = nc.dram_tensor("bvb", [128, HPC * DH], F32, kind="ExternalInput").ap()
    wo_d = nc.dram_tensor("wo", [128, NP * 1024], BF16, kind="ExternalInput").ap()
    sel_d = nc.dram_tensor("sel", [128, NP * 128], BF16, kind="ExternalInput").ap()
    tri_d = nc.dram_tensor("tri", [128, 128], BF16, kind="ExternalInput").ap()
    out_d = nc.dram_tensor("out_s", [S, D], BF16, kind="ExternalOutput").ap()
    io = (xt_d, wqk_d, bqk_d, wv_d, bv_d, wo_d, sel_d, tri_d, out_d)
    with tile.TileContext(nc) as tc:
        with ExitStack() as ctx:
            _emit(ctx, tc, io)
    nc.compile()
    return nc


_NC = None


def _get_nc():
    global _NC
    if _NC is None:
        _NC = _build()
    return _NC


def _host_inputs(x, w_qkv, b_qkv, w_out):
    """Per-head-group shared weight arrays + per-core x, pre-arranged into
    SBUF layouts so every load is one large contiguous DMA."""
    import ml_dtypes
    maps = []
    hg_arrs = []
    for hg in range(2):
        hs = slice(hg * HPC, (hg + 1) * HPC)
        wq = np.asarray(w_qkv[:, 0, hs, :]).reshape(D, HPC * DH)
        wk = np.asarray(w_qkv[:, 1, hs, :]).reshape(D, HPC * DH)
        wqk_cat = np.concatenate([wq, wk], axis=1)  # [D, 2*HPC*DH], col mt*128+c
        # wqk chunked: [128, (mt, dc, 128)]
        wqk_arr = np.zeros((128, 8 * 1024), np.float32)
        for mt in range(8):
            for dc in range(NDC):
                wqk_arr[:, mt * 1024 + dc * 128:(mt * 1024 + dc * 128) + 128] = (
                    wqk_cat[dc * 128:(dc + 1) * 128, mt * 128:(mt + 1) * 128])
        bq = np.asarray(b_qkv[0, hs, :]).reshape(HPC * DH)
        bk = np.asarray(b_qkv[1, hs, :]).reshape(HPC * DH)
        bqk = np.zeros((128, 8), np.float32)
        for mt in range(8):
            t, j = mt // NP, mt % NP
            src = bq if t == 0 else bk
            bqk[:, mt] = src[j * 128:(j + 1) * 128]
        wv = np.asarray(w_qkv[:, 2, hs, :]).reshape(D, HPC * DH)
        # wv chunked: [128, (dc, 512)]
        wv_arr = np.zeros((128, NDC * 512), np.float32)
        for dc in range(NDC):
            wv_arr[:, dc * 512:(dc + 1) * 512] = wv[dc * 128:(dc + 1) * 128, :]
        bvb = np.broadcast_to(
            np.asarray(b_qkv[2, hs, :]).reshape(1, HPC * DH), (128, HPC * DH)
        ).astype(np.float32)
        wo = np.asarray(w_out[hs]).reshape(HPC * DH, D)
        # wo chunked: [128, (j, 1024)]
        wo_arr = np.zeros((128, NP * 1024), np.float32)
        for j in range(NP):
            wo_arr[:, j * 1024:(j + 1) * 1024] = wo[j * 128:(j + 1) * 128, :]
        selm = np.zeros((128, NP * 128), ml_dtypes.bfloat16)
        for j in range(NP):
            for p in range(128):
                selm[32 * j + p // 64, j * 128 + p] = 1.0
        trim = (np.arange(128)[None, :] >= np.arange(128)[:, None]).astype(
            ml_dtypes.bfloat16)
        hg_arrs.append(dict(
            wqk=np.ascontiguousarray(wqk_arr).astype(ml_dtypes.bfloat16),
            bqk=bqk,
            wv=np.ascontiguousarray(wv_arr).astype(ml_dtypes.bfloat16),
            bvb=bvb,
            wo=np.ascontiguousarray(wo_arr).astype(ml_dtypes.bfloat16),
            sel=selm, tri=trim))
    for c in range(8):
        b, hg = c % B, c // B
        m = dict(hg_arrs[hg])
        xb = np.asarray(x[b]).astype(ml_dtypes.bfloat16)  # [S, D]
        # xt chunked: [128, (dc, S)]: xt[p, dc*S + s] = x[s, dc*128+p]
        xtc = np.zeros((128, NDC * S), ml_dtypes.bfloat16)
        for dc in range(NDC):
            xtc[:, dc * S:(dc + 1) * S] = xb[:, dc * 128:(dc + 1) * 128].T
        m["xt_s"] = np.ascontiguousarray(xtc)
        maps.append(m)
    return maps


def _run(inputs, trace=False, tmpdir=None):
    nc = _get_nc()
    in_maps = _host_inputs(inputs["x"], inputs["w_qkv"], inputs["b_qkv"],
                           inputs["w_out"])
    res = bass_utils.run_bass_kernel_spmd(
        nc, in_maps, core_ids=list(range(8)), trace=trace, tmpdir=tmpdir)
    b_out = np.asarray(inputs["b_out"], dtype=np.float32)
    out = np.empty((B, S, D), np.float32)
    for b in range(B):
        out[b] = (res.results[b]["out_s"].astype(np.float32)
                  + res.results[b + B]["out_s"].astype(np.float32)
                  + b_out[None, :])
    return out, res


def kernel(**inputs) -> np.ndarray:
    out, _ = _run(inputs, trace=False)
    return out
